# revision 19
# baseline (speedup 1.0000x reference)
"""Trainium2 Bass kernel for nn_ClassifyMLPHeadForKCRWithConcatChoices.

Math (B=16, L=2048, H=A=1024, C=5):
  keys  = tanh(X @ Wh^T + bh)                    (B,L,A)
  probs = keys @ (q / sqrt(A*var(q)))            (B,L)
  z     = probs * (-1000 * (1 - attn))           (B,L)
  att   = softmax_L(z)                           (B,L)
  vals  = att[...,None] + X                      (B,L,H)
  ctx   = einsum('bcl,blh->bch', seg, vals)
  logit = ctx @ Wc^T + bc                        (B,C,1)

Structural facts that eliminate nearly all of the FLOPs:

1. att broadcasts over H and the classifier is rank-1:
     logit[b,c] = S_att[b,c]*sum(Wc) + Sy[b,c] + bc
   with S_att = seg-pooled att and Sy = seg-pooled y, y = X @ Wc.

2. The softmax logits are probs * mask with mask = -1000*(1-attn):
   z == 0 at every attended token and z = -1000*probs at padded
   tokens.  probs has std ~0.55, so each row's max logit is
   ~1000*|min probs| >= several hundred, while every segment token
   (segments are subsets of the attended region) has logit 0.  In
   fp32, exp(0 - z_max) underflows to exactly 0 once z_max > ~104, so
   the reference's own softmax gives att == 0 at every segment token
   and S_att == 0 identically.  The keys/probs pipeline is dead code
   for any row that has one padded token with probs <= -0.04 (z_max
   >= 40 makes S_att < 1e-14).  The host proves this per row by
   sampling a few padded-token probs (~1 GFLOP on 512 tokens); rows
   that fail the test (none do for this data regime) fall back to an
   exact host softmax, and rows with no padding at all have z == 0
   everywhere -> exactly uniform att, no probs needed.

So the only computation whose value reaches the output is the rank-1
classifier projection y = X @ Wc over segment tokens.  The device
computes exactly that: X^T for the ~28.5K segment tokens is gathered
and split evenly across the 8 cores (weights replicated, per the
data-parallel hint), and each core runs a col-tiled rank-1 matmul:

  - 4 concurrent PE column tiles (tile_position=(0,32j)) each
    contract 2 of the 8 128-row h-blocks per 512-token chunk: 1024
    PE cycles/chunk, ~2 cycles/token -- the bf16 rhs-streaming floor.
  - PSUM evacuation rotates across the DVE, Pool and ACT engines so
    no single engine paces the loop (a lone DVE copy chain would).
  - per-chunk output DMAs alternate between the two HW DGE queues
    (SP, ACT); each moves only 1KB per partition line, keeping both
    queues far under the PE time.

The 4 column-tile partials (partitions 0/32/64/96), segment pooling
and the rank-1 recombination run on the host during unsharding.
"""

import sys

if '/opt/trn_rl_repo' not in sys.path:
    sys.path.insert(0, '/opt/trn_rl_repo')

import numpy as np
import ml_dtypes

import concourse.bass as bass  # noqa: F401  (bass must import before bacc)
import concourse.mybir as mybir
import concourse.tile as tile
from concourse import bacc
from concourse.bass_utils import run_bass_kernel_spmd

B, L, H, A, C = 16, 2048, 1024, 1024, 5
N_CORES = 8
P = 128
HB = H // P                 # contraction blocks
CH = 512                    # token chunk (one PSUM bank)
YCAP = 3584                 # compact segment-token capacity per core
MAX_YCH = 8                 # SBUF cap: 8 chunks = 4096 tokens/core

BF16 = mybir.dt.bfloat16
FP32 = mybir.dt.float32


def build_program(repeat: int = 1, n_cores: int = N_CORES, ycap: int = YCAP,
                  stages: str = "full"):
    """Col-tiled rank-1 classifier projection y = X @ Wc over ycap
    gathered segment tokens per core.

    All ych (<= 8) chunks are packed into a single PSUM bank: chunk k
    goes through an 8-wide lhsT whose only nonzero column is k, so its
    partials land on partition lines 32j+k of the shared bank (zero
    columns accumulate 0 onto the other chunks' lines).  One
    evacuation copy and one DMA per iteration then move the whole
    bank, taking both far off the critical path (an engine copy costs
    ~free-size regardless of partitions, and DMA time is
    per-partition-line bytes, so 128 lines cost the same as 4).

    stages: "full" | "mm" (timing experiment: matmuls only).
    """
    assert ycap % CH == 0
    ych = ycap // CH
    assert ych <= 8
    nc = bacc.Bacc("TRN2", target_bir_lowering=False, debug=False,
                   num_devices=n_cores)
    xt_d = nc.dram_tensor("xt", [HB, P, ycap], BF16, kind="ExternalInput")
    # 8 shifted weight variants: wp[p, hb, k, c] = Wc[hb*128+p] if c == k
    wp_d = nc.dram_tensor("wp", [P, HB * 8 * 8], BF16, kind="ExternalInput")
    # y partials leave as bf16: their rounding (~0.2% on a term that is
    # itself bf16-limited) is invisible next to the 2e-2 gate.  The
    # full 128-partition evacuation tile is DMA'd; the host reads
    # lines 32j+k and ignores the rest.
    y4_d = nc.dram_tensor("y4", [P, CH], BF16, kind="ExternalOutput")

    with tile.TileContext(nc) as tc:
        with (
            tc.tile_pool(name="const", bufs=1) as const,
            tc.tile_pool(name="xpool", bufs=1) as xpool,
            tc.tile_pool(name="vecs", bufs=4) as vecs,
            tc.tile_pool(name="ps_y", bufs=4, space="PSUM") as ps_y,
        ):
            wp_sb = const.tile([P, HB, 8, 8], BF16)
            nc.sync.dma_start(
                wp_sb[:],
                wp_d.ap().rearrange("p (h k s) -> p h k s", h=HB, k=8))
            # warm the ACT copy path during the input DMA window so the
            # first in-loop ACT evacuation doesn't pay a table load
            warm = const.tile([1, 1], BF16)
            nc.scalar.copy(warm[:], wp_sb[:1, 0, 0, 0:1])

            # X^T staged per (hb, chunk)
            xt_sb = {}
            for ch in range(ych):
                for hb in range(HB):
                    t = xpool.tile([P, CH], BF16, tag=f"x{hb}_{ch}")
                    nc.sync.dma_start(
                        t[:], xt_d.ap()[hb, :, ch * CH:(ch + 1) * CH])
                    xt_sb[hb, ch] = t

            for it in range(repeat):
                py = ps_y.tile([P, CH], FP32, tag="py")
                if it < 4:
                    # first use of each ring bank: zero the lines the
                    # matmuls never touch so the full-width DMA ships
                    # only finite values (power-on PSUM may hold NaN)
                    nc.vector.memset(py[:], 0.0)
                for k in range(ych):
                    for r in range(2):
                        for j in range(4):
                            hb = 4 * r + j
                            nc.tensor.matmul(
                                py[32 * j:32 * j + ych, :],
                                lhsT=wp_sb[:, hb, k, 0:ych],
                                rhs=xt_sb[hb, k][:],
                                start=(k == 0 and r == 0),
                                stop=(k == ych - 1 and r == 1),
                                tile_position=(0, 32 * j),
                            )
                if stages == "mm":
                    continue
                ysg = vecs.tile([P, CH], BF16, tag="ysg")
                # one evacuation (DVE; Pool/GPSIMD cannot access PSUM)
                # and one full-width DMA per iteration; line 32j+k is
                # chunk k's partial j
                nc.vector.tensor_copy(ysg[:], py[:])
                nc.sync.dma_start(y4_d.ap(), ysg[:])

    nc.compile()
    return nc


def _seg_mask(attn, mlm):
    """(B, C, L) segment mask, exactly as the reference builds it."""
    idx = np.arange(L)
    marker = np.where(mlm > 0, idx[None, :], L)
    starts = np.sort(marker, axis=1)[:, :C]
    end_idx = attn.sum(axis=1)
    bounds = np.concatenate([starts[:, 1:] - 1, (end_idx - 1)[:, None]],
                            axis=1)
    return ((idx[None, None, :] >= starts[:, :, None] + 1)
            & (idx[None, None, :] < bounds[:, :, None]))


def _host_att(X, attn, seg, Wh, bh, q, scale):
    """Per-row seg-pooled attention S_att (B, C).

    Saturated rows (one padded token with probs <= -0.04, i.e. max
    logit >= 40 vs the segment tokens' 0) have S_att < 1e-14 -> 0.
    Rows with no padding have z == 0 everywhere -> uniform att.  Any
    other row gets an exact host softmax.
    """
    S_att = np.zeros((B, C), np.float32)
    for b in range(B):
        pad = np.nonzero(attn[b] == 0)[0]
        if len(pad) == 0:
            S_att[b] = seg[b].sum(axis=1, dtype=np.float64) / L
            continue
        samp = pad[:32]
        pr = np.tanh(X[b, samp] @ Wh.T + bh) @ q * scale
        if pr.min() <= -0.04:
            continue  # saturated: S_att stays 0
        pr = np.tanh(X[b, pad] @ Wh.T + bh) @ q * scale
        z = np.zeros(L)
        z[pad] = -1000.0 * pr
        z -= z.max()
        e = np.exp(z)
        S_att[b] = (seg[b] @ (e / e.sum())).astype(np.float32)
    return S_att


def prep_inputs(inputs):
    """Full inputs -> (per-core in_maps, host epilogue context)."""
    X = np.ascontiguousarray(np.asarray(inputs["input"], dtype=np.float32))
    attn = np.asarray(inputs["attention_mask"])
    mlm = np.asarray(inputs["mlm_mask"])
    Wh = np.asarray(inputs["W_hidden"], dtype=np.float32)
    bh = np.asarray(inputs["b_hidden"], dtype=np.float32)
    q = np.asarray(inputs["query"], dtype=np.float32)[:, 0]
    Wc = np.asarray(inputs["W_cls"], dtype=np.float32)[0]
    bc = float(np.asarray(inputs["b_cls"], dtype=np.float32)[0])

    qvar = np.var(q.astype(np.float64), ddof=1)
    scale = 1.0 / np.sqrt(A * qvar)

    wcT = Wc.reshape(HB, P).T                      # [P, HB]
    wp = np.zeros((P, HB, 8, 8), np.float32)       # shifted variants
    for k in range(8):
        wp[:, :, k, k] = wcT
    wp = wp.reshape(P, HB * 8 * 8).astype(ml_dtypes.bfloat16)

    # --- compact gather of segment tokens (the only ones Sy pools) ---
    seg = _seg_mask(attn, mlm)
    need_y = seg.any(axis=1)
    yb, yt = np.nonzero(need_y)
    t_y = len(yb)
    ycap = CH * max(1, min(MAX_YCH, int(np.ceil(t_y / (N_CORES * CH)))))
    n_ydev = min(t_y, N_CORES * ycap)
    yslots = np.zeros(N_CORES * ycap, np.int64)
    yslots[:n_ydev] = (yb * L + yt)[:n_ydev]

    Xf = X.reshape(B * L, H)
    in_maps = []
    for c in range(N_CORES):
        ycols = yslots[c * ycap:(c + 1) * ycap]
        # xt[hb, p, t] = X[ycols[t], hb*128+p]
        xt_c = np.ascontiguousarray(
            Xf[ycols].T.reshape(HB, P, ycap)).astype(ml_dtypes.bfloat16)
        in_maps.append(dict(xt=xt_c, wp=wp))

    S_att = _host_att(X, attn, seg, Wh, bh, q, scale)
    host_ctx = dict(seg=seg, S_att=S_att, Wc=Wc, bc=bc, X=X, ycap=ycap,
                    n_ydev=n_ydev, yb=yb, yt=yt)
    return in_maps, host_ctx


def epilogue(y, ctx):
    """Host: segment pooling + rank-1 classifier recombination."""
    seg = ctx["seg"].astype(np.float32)
    Sy = np.einsum("bcl,bl->bc", seg, y)
    S_att = ctx["S_att"]
    Wsum = ctx["Wc"].sum(dtype=np.float32)
    return (S_att * Wsum + Sy + ctx["bc"]).astype(np.float32)[:, :, None]


_prog_cache = {}


def kernel(**inputs) -> np.ndarray:
    in_maps, ctx = prep_inputs(inputs)
    key = (ctx["ycap"],)
    if key not in _prog_cache:
        _prog_cache[key] = build_program(ycap=ctx["ycap"])
    nc = _prog_cache[key]
    res = run_bass_kernel_spmd(nc, in_maps, core_ids=list(range(N_CORES)))
    ych = ctx["ycap"] // CH
    percore = []
    for c in range(N_CORES):
        y4g = res.results[c]["y4"]            # [128, CH] bf16
        # line 32j+k = tile-j partial of chunk k
        g4 = y4g.reshape(4, 32, CH).astype(np.float32)
        yc = g4[:, 0:ych, :].sum(axis=0)      # [ych, CH] over tiles
        percore.append(yc.reshape(-1))
    yflat = np.concatenate(percore)
    n_ydev, yb, yt = ctx["n_ydev"], ctx["yb"], ctx["yt"]
    y = np.zeros((B, L), np.float32)
    y[yb[:n_ydev], yt[:n_ydev]] = yflat[:n_ydev]
    if n_ydev < len(yb):  # y leftover beyond device capacity, on host
        lb, lt = yb[n_ydev:], yt[n_ydev:]
        y[lb, lt] = (ctx["X"][lb, lt] @ ctx["Wc"]).astype(np.float32)
    return epilogue(y, ctx)


# revision 23
# speedup vs baseline: 1.0776x; 1.0776x over previous
"""Trainium2 Bass kernel for nn_ClassifyMLPHeadForKCRWithConcatChoices.

Math (B=16, L=2048, H=A=1024, C=5):
  keys  = tanh(X @ Wh^T + bh)                    (B,L,A)
  probs = keys @ (q / sqrt(A*var(q)))            (B,L)
  z     = probs * (-1000 * (1 - attn))           (B,L)
  att   = softmax_L(z)                           (B,L)
  vals  = att[...,None] + X                      (B,L,H)
  ctx   = einsum('bcl,blh->bch', seg, vals)
  logit = ctx @ Wc^T + bc                        (B,C,1)

Structural facts that eliminate nearly all of the FLOPs:

1. att broadcasts over H and the classifier is rank-1:
     logit[b,c] = S_att[b,c]*sum(Wc) + Sy[b,c] + bc
   with S_att = seg-pooled att and Sy = seg-pooled y, y = X @ Wc.

2. The softmax logits are probs * mask with mask = -1000*(1-attn):
   z == 0 at every attended token and z = -1000*probs at padded
   tokens.  probs has std ~0.55, so each row's max logit is
   ~1000*|min probs| >= several hundred, while every segment token
   (segments are subsets of the attended region) has logit 0.  In
   fp32, exp(0 - z_max) underflows to exactly 0 once z_max > ~104, so
   the reference's own softmax gives att == 0 at every segment token
   and S_att == 0 identically.  The keys/probs pipeline is dead code
   for any row that has one padded token with probs <= -0.04 (z_max
   >= 40 makes S_att < 1e-14).  The host proves this per row by
   sampling a few padded-token probs (~1 GFLOP on 512 tokens); rows
   that fail the test (none do for this data regime) fall back to an
   exact host softmax, and rows with no padding at all have z == 0
   everywhere -> exactly uniform att, no probs needed.

So the only computation whose value reaches the output is the rank-1
classifier projection y = X @ Wc over segment tokens.  The device
computes exactly that: X^T for the ~28.5K segment tokens is gathered
and split evenly across the 8 cores (weights replicated, per the
data-parallel hint), and each core runs a col-tiled rank-1 matmul:

  - 4 concurrent PE column tiles (tile_position=(0,32j)) each
    contract 2 of the 8 128-row h-blocks per 512-token chunk: 1024
    PE cycles/chunk, ~2 cycles/token -- the bf16 rhs-streaming floor.
  - PSUM evacuation rotates across the DVE, Pool and ACT engines so
    no single engine paces the loop (a lone DVE copy chain would).
  - per-chunk output DMAs alternate between the two HW DGE queues
    (SP, ACT); each moves only 1KB per partition line, keeping both
    queues far under the PE time.

The 4 column-tile partials (partitions 0/32/64/96), segment pooling
and the rank-1 recombination run on the host during unsharding.
"""

import sys

if '/opt/trn_rl_repo' not in sys.path:
    sys.path.insert(0, '/opt/trn_rl_repo')

import numpy as np
import ml_dtypes

import concourse.bass as bass  # noqa: F401  (bass must import before bacc)
import concourse.mybir as mybir
import concourse.tile as tile
from concourse import bacc
from concourse.bass_utils import run_bass_kernel_spmd

B, L, H, A, C = 16, 2048, 1024, 1024, 5
N_CORES = 8
P = 128
HB = H // P                 # contraction blocks
CH = 512                    # token chunk (one PSUM bank)
YCAP = 3584                 # compact segment-token capacity per core
MAX_YCH = 8                 # SBUF cap: 8 chunks = 4096 tokens/core

BF16 = mybir.dt.bfloat16
FP32 = mybir.dt.float32


def build_program(repeat: int = 1, n_cores: int = N_CORES, ycap: int = YCAP,
                  stages: str = "full"):
    """Col-tiled rank-1 classifier projection y = X @ Wc over ycap
    gathered segment tokens per core.

    Chunks are packed 4-per-PSUM-bank: chunk k of a group goes through
    a 4-wide lhsT whose only nonzero column is k, so its partials land
    on partition lines 32j+k of the shared bank (zero columns
    accumulate 0 onto the other chunks' lines).  One evacuation copy
    and one full-width DMA per group then replace per-chunk ops,
    taking both far off the critical path (an engine copy costs
    ~free-size regardless of partitions, and DMA time is
    per-partition-line bytes, so 128 lines cost the same as 4).
    Wider packing (7 chunks in one bank) measured slower — the wider
    lhsT costs the PE more than the saved evacuation.

    stages: "full" | "mm" (timing experiment: matmuls only).
    """
    assert ycap % CH == 0
    ych = ycap // CH
    groups = [list(range(g, min(g + 4, ych))) for g in range(0, ych, 4)]
    nc = bacc.Bacc("TRN2", target_bir_lowering=False, debug=False,
                   num_devices=n_cores)
    xt_d = nc.dram_tensor("xt", [HB, P, ycap], BF16, kind="ExternalInput")
    # 8 shifted weight variants: wp[p, hb, k, c] = Wc[hb*128+p] if c == k
    wp_d = nc.dram_tensor("wp", [P, HB * 8 * 8], BF16, kind="ExternalInput")
    # y partials leave as bf16: their rounding (~0.2% on a term that is
    # itself bf16-limited) is invisible next to the 2e-2 gate.  The
    # full 128-partition evacuation tile is DMA'd per group; the host
    # reads lines 32j+k and ignores the rest.
    ngrp = len(groups)
    y4_d = nc.dram_tensor("y4", [ngrp, P, CH], BF16, kind="ExternalOutput")

    with tile.TileContext(nc) as tc:
        with (
            tc.tile_pool(name="const", bufs=1) as const,
            tc.tile_pool(name="xpool", bufs=1) as xpool,
            tc.tile_pool(name="vecs", bufs=4) as vecs,
            tc.tile_pool(name="ps_y", bufs=4, space="PSUM") as ps_y,
        ):
            wp_sb = const.tile([P, HB, 8, 8], BF16)
            nc.sync.dma_start(
                wp_sb[:],
                wp_d.ap().rearrange("p (h k s) -> p h k s", h=HB, k=8))
            # warm the ACT copy path during the input DMA window so the
            # first in-loop ACT evacuation doesn't pay a table load
            warm = const.tile([1, 1], BF16)
            nc.scalar.copy(warm[:], wp_sb[:1, 0, 0, 0:1])

            # X^T staged per (hb, chunk)
            xt_sb = {}
            for ch in range(ych):
                for hb in range(HB):
                    t = xpool.tile([P, CH], BF16, tag=f"x{hb}_{ch}")
                    nc.sync.dma_start(
                        t[:], xt_d.ap()[hb, :, ch * CH:(ch + 1) * CH])
                    xt_sb[hb, ch] = t

            ai = 0
            for _ in range(repeat):
                for gi, grp in enumerate(groups):
                    py = ps_y.tile([P, CH], FP32, tag="py")
                    if ai < 4:
                        # first use of each ring bank: zero the lines
                        # the matmuls never touch so the full-width DMA
                        # ships only finite values (power-on PSUM may
                        # hold NaN)
                        nc.vector.memset(py[:], 0.0)
                    ai += 1
                    for k, ch in enumerate(grp):
                        for r in range(2):
                            for j in range(4):
                                hb = 4 * r + j
                                nc.tensor.matmul(
                                    py[32 * j:32 * j + 4, :],
                                    lhsT=wp_sb[:, hb, k, 0:4],
                                    rhs=xt_sb[hb, ch][:],
                                    start=(k == 0 and r == 0),
                                    stop=(k == len(grp) - 1 and r == 1),
                                    tile_position=(0, 32 * j),
                                )
                    if stages == "mm":
                        continue
                    ysg = vecs.tile([P, CH], BF16, tag="ysg")
                    # one evacuation per group, alternating DVE/ACT
                    # (Pool/GPSIMD cannot access PSUM), and one
                    # full-width DMA per group on alternating HW DGE
                    # queues; line 32j+k is chunk grp[k]'s partial j
                    if gi % 2 == 0:
                        nc.vector.tensor_copy(ysg[:], py[:])
                    else:
                        nc.scalar.copy(ysg[:], py[:])
                    dq = nc.sync if gi % 2 == 0 else nc.scalar
                    dq.dma_start(y4_d.ap()[gi, :, :], ysg[:])

    nc.compile()
    return nc


def _seg_mask(attn, mlm):
    """(B, C, L) segment mask, exactly as the reference builds it."""
    idx = np.arange(L)
    marker = np.where(mlm > 0, idx[None, :], L)
    starts = np.sort(marker, axis=1)[:, :C]
    end_idx = attn.sum(axis=1)
    bounds = np.concatenate([starts[:, 1:] - 1, (end_idx - 1)[:, None]],
                            axis=1)
    return ((idx[None, None, :] >= starts[:, :, None] + 1)
            & (idx[None, None, :] < bounds[:, :, None]))


def _host_att(X, attn, seg, Wh, bh, q, scale):
    """Per-row seg-pooled attention S_att (B, C).

    Saturated rows (one padded token with probs <= -0.04, i.e. max
    logit >= 40 vs the segment tokens' 0) have S_att < 1e-14 -> 0.
    Rows with no padding have z == 0 everywhere -> uniform att.  Any
    other row gets an exact host softmax.
    """
    S_att = np.zeros((B, C), np.float32)
    for b in range(B):
        pad = np.nonzero(attn[b] == 0)[0]
        if len(pad) == 0:
            S_att[b] = seg[b].sum(axis=1, dtype=np.float64) / L
            continue
        samp = pad[:32]
        pr = np.tanh(X[b, samp] @ Wh.T + bh) @ q * scale
        if pr.min() <= -0.04:
            continue  # saturated: S_att stays 0
        pr = np.tanh(X[b, pad] @ Wh.T + bh) @ q * scale
        z = np.zeros(L)
        z[pad] = -1000.0 * pr
        z -= z.max()
        e = np.exp(z)
        S_att[b] = (seg[b] @ (e / e.sum())).astype(np.float32)
    return S_att


def prep_inputs(inputs):
    """Full inputs -> (per-core in_maps, host epilogue context)."""
    X = np.ascontiguousarray(np.asarray(inputs["input"], dtype=np.float32))
    attn = np.asarray(inputs["attention_mask"])
    mlm = np.asarray(inputs["mlm_mask"])
    Wh = np.asarray(inputs["W_hidden"], dtype=np.float32)
    bh = np.asarray(inputs["b_hidden"], dtype=np.float32)
    q = np.asarray(inputs["query"], dtype=np.float32)[:, 0]
    Wc = np.asarray(inputs["W_cls"], dtype=np.float32)[0]
    bc = float(np.asarray(inputs["b_cls"], dtype=np.float32)[0])

    qvar = np.var(q.astype(np.float64), ddof=1)
    scale = 1.0 / np.sqrt(A * qvar)

    wcT = Wc.reshape(HB, P).T                      # [P, HB]
    wp = np.zeros((P, HB, 8, 8), np.float32)       # shifted variants
    for k in range(8):
        wp[:, :, k, k] = wcT
    wp = wp.reshape(P, HB * 8 * 8).astype(ml_dtypes.bfloat16)

    # --- compact gather of segment tokens (the only ones Sy pools) ---
    seg = _seg_mask(attn, mlm)
    need_y = seg.any(axis=1)
    yb, yt = np.nonzero(need_y)
    t_y = len(yb)
    ycap = CH * max(1, min(MAX_YCH, int(np.ceil(t_y / (N_CORES * CH)))))
    n_ydev = min(t_y, N_CORES * ycap)
    yslots = np.zeros(N_CORES * ycap, np.int64)
    yslots[:n_ydev] = (yb * L + yt)[:n_ydev]

    Xf = X.reshape(B * L, H)
    in_maps = []
    for c in range(N_CORES):
        ycols = yslots[c * ycap:(c + 1) * ycap]
        # xt[hb, p, t] = X[ycols[t], hb*128+p]
        xt_c = np.ascontiguousarray(
            Xf[ycols].T.reshape(HB, P, ycap)).astype(ml_dtypes.bfloat16)
        in_maps.append(dict(xt=xt_c, wp=wp))

    S_att = _host_att(X, attn, seg, Wh, bh, q, scale)
    host_ctx = dict(seg=seg, S_att=S_att, Wc=Wc, bc=bc, X=X, ycap=ycap,
                    n_ydev=n_ydev, yb=yb, yt=yt)
    return in_maps, host_ctx


def epilogue(y, ctx):
    """Host: segment pooling + rank-1 classifier recombination."""
    seg = ctx["seg"].astype(np.float32)
    Sy = np.einsum("bcl,bl->bc", seg, y)
    S_att = ctx["S_att"]
    Wsum = ctx["Wc"].sum(dtype=np.float32)
    return (S_att * Wsum + Sy + ctx["bc"]).astype(np.float32)[:, :, None]


_prog_cache = {}


def kernel(**inputs) -> np.ndarray:
    in_maps, ctx = prep_inputs(inputs)
    key = (ctx["ycap"],)
    if key not in _prog_cache:
        _prog_cache[key] = build_program(ycap=ctx["ycap"])
    nc = _prog_cache[key]
    res = run_bass_kernel_spmd(nc, in_maps, core_ids=list(range(N_CORES)))
    ych = ctx["ycap"] // CH
    percore = []
    for c in range(N_CORES):
        y4g = res.results[c]["y4"]            # [ngrp, 128, CH] bf16
        # line 32j+b of group g = tile-j partial of chunk 4g+b
        g4 = y4g.reshape(y4g.shape[0], 4, 32, CH).astype(np.float32)
        yc = g4[:, :, 0:4, :].sum(axis=1)     # [ngrp, 4, CH] over tiles
        percore.append(yc.reshape(-1, CH)[:ych].reshape(-1))
    yflat = np.concatenate(percore)
    n_ydev, yb, yt = ctx["n_ydev"], ctx["yb"], ctx["yt"]
    y = np.zeros((B, L), np.float32)
    y[yb[:n_ydev], yt[:n_ydev]] = yflat[:n_ydev]
    if n_ydev < len(yb):  # y leftover beyond device capacity, on host
        lb, lt = yb[n_ydev:], yt[n_ydev:]
        y[lb, lt] = (ctx["X"][lb, lt] @ ctx["Wc"]).astype(np.float32)
    return epilogue(y, ctx)


# revision 27
# speedup vs baseline: 1.6526x; 1.5336x over previous
"""Trainium2 Bass kernel for nn_ClassifyMLPHeadForKCRWithConcatChoices.

Math (B=16, L=2048, H=A=1024, C=5):
  keys  = tanh(X @ Wh^T + bh)                    (B,L,A)
  probs = keys @ (q / sqrt(A*var(q)))            (B,L)
  z     = probs * (-1000 * (1 - attn))           (B,L)
  att   = softmax_L(z)                           (B,L)
  vals  = att[...,None] + X                      (B,L,H)
  ctx   = einsum('bcl,blh->bch', seg, vals)
  logit = ctx @ Wc^T + bc                        (B,C,1)

Structural facts that eliminate nearly all of the FLOPs:

1. att broadcasts over H and the classifier is rank-1:
     logit[b,c] = S_att[b,c]*sum(Wc) + Sy[b,c] + bc
   with S_att = seg-pooled att and Sy = seg-pooled y, y = X @ Wc.

2. The softmax logits are probs * mask with mask = -1000*(1-attn):
   z == 0 at every attended token and z = -1000*probs at padded
   tokens.  probs has std ~0.55, so each row's max logit is
   ~1000*|min probs| >= several hundred, while every segment token
   (segments are subsets of the attended region) has logit 0.  In
   fp32, exp(0 - z_max) underflows to exactly 0 once z_max > ~104, so
   the reference's own softmax gives att == 0 at every segment token
   and S_att == 0 identically.  The keys/probs pipeline is dead code
   for any row that has one padded token with probs <= -0.04 (z_max
   >= 40 makes S_att < 1e-14).  The host proves this per row by
   sampling a few padded-token probs (~1 GFLOP on 512 tokens); rows
   that fail the test (none do for this data regime) fall back to an
   exact host softmax, and rows with no padding at all have z == 0
   everywhere -> exactly uniform att, no probs needed.

So the only computation whose value reaches the output is the rank-1
classifier projection y = X @ Wc over segment tokens.  The device
computes exactly that: X^T for the ~28.5K segment tokens is gathered
and split evenly across the 8 cores (weights replicated, per the
data-parallel hint), and each core runs a col-tiled rank-1 matmul:

  - 4 concurrent PE column tiles (tile_position=(0,32j)) each
    contract 2 of the 8 128-row h-blocks per 512-token chunk: 1024
    PE cycles/chunk, ~2 cycles/token -- the bf16 rhs-streaming floor.
  - PSUM evacuation rotates across the DVE, Pool and ACT engines so
    no single engine paces the loop (a lone DVE copy chain would).
  - per-chunk output DMAs alternate between the two HW DGE queues
    (SP, ACT); each moves only 1KB per partition line, keeping both
    queues far under the PE time.

The 4 column-tile partials (partitions 0/32/64/96), segment pooling
and the rank-1 recombination run on the host during unsharding.
"""

import sys

if '/opt/trn_rl_repo' not in sys.path:
    sys.path.insert(0, '/opt/trn_rl_repo')

import numpy as np
import ml_dtypes

import concourse.bass as bass  # noqa: F401  (bass must import before bacc)
import concourse.mybir as mybir
import concourse.tile as tile
from concourse import bacc
from concourse.bass_utils import run_bass_kernel_spmd

B, L, H, A, C = 16, 2048, 1024, 1024, 5
N_CORES = 8
P = 128
HB = H // P                 # contraction blocks
CH = 512                    # token chunk (one PSUM bank)
YCAP = 3584                 # compact segment-token capacity per core
MAX_YCH = 8                 # SBUF cap: 8 chunks = 4096 tokens/core

BF16 = mybir.dt.bfloat16
FP32 = mybir.dt.float32


def build_program(repeat: int = 1, n_cores: int = N_CORES, ycap: int = YCAP,
                  stages: str = "full", gsz: int = 4):
    """Col-tiled rank-1 classifier projection y = X @ Wc over ycap
    gathered segment tokens per core.

    Chunks are packed 4-per-PSUM-bank: chunk k of a group goes through
    a 4-wide lhsT whose only nonzero column is k, so its partials land
    on partition lines 32j+k of the shared bank (zero columns
    accumulate 0 onto the other chunks' lines).  One evacuation copy
    and one full-width DMA per group then replace per-chunk ops,
    taking both far off the critical path (an engine copy costs
    ~free-size regardless of partitions, and DMA time is
    per-partition-line bytes, so 128 lines cost the same as 4).
    Wider packing (7 chunks in one bank) measured slower — the wider
    lhsT costs the PE more than the saved evacuation.

    stages: "full" | "mm" (timing experiment: matmuls only).
    """
    assert ycap % CH == 0
    ych = ycap // CH
    groups = [list(range(g, min(g + gsz, ych))) for g in range(0, ych, gsz)]
    nc = bacc.Bacc("TRN2", target_bir_lowering=False, debug=False,
                   num_devices=n_cores)
    xt_d = nc.dram_tensor("xt", [HB, P, ycap], BF16, kind="ExternalInput")
    # 8 shifted weight variants: wp[p, hb, k, c] = Wc[hb*128+p] if c == k
    wp_d = nc.dram_tensor("wp", [P, HB * 8 * 8], BF16, kind="ExternalInput")
    # y partials leave as bf16: their rounding (~0.2% on a term that is
    # itself bf16-limited) is invisible next to the 2e-2 gate.  The
    # full 128-partition evacuation tile is DMA'd per group; the host
    # reads lines 32j+k and ignores the rest.
    ngrp = len(groups)
    y4_d = nc.dram_tensor("y4", [ngrp, P, CH], BF16, kind="ExternalOutput")

    with tile.TileContext(nc) as tc:
        with (
            tc.tile_pool(name="const", bufs=1) as const,
            tc.tile_pool(name="xpool", bufs=1) as xpool,
            tc.tile_pool(name="vecs", bufs=4) as vecs,
            tc.tile_pool(name="ps_y", bufs=4, space="PSUM") as ps_y,
        ):
            wp_sb = const.tile([P, HB, 8, 8], BF16)
            nc.sync.dma_start(
                wp_sb[:],
                wp_d.ap().rearrange("p (h k s) -> p h k s", h=HB, k=8))
            # warm the ACT copy path during the input DMA window so the
            # first in-loop ACT evacuation doesn't pay a table load
            warm = const.tile([1, 1], BF16)
            nc.scalar.copy(warm[:], wp_sb[:1, 0, 0, 0:1])

            # X^T staged per (hb, chunk)
            xt_sb = {}
            for ch in range(ych):
                for hb in range(HB):
                    t = xpool.tile([P, CH], BF16, tag=f"x{hb}_{ch}")
                    nc.sync.dma_start(
                        t[:], xt_d.ap()[hb, :, ch * CH:(ch + 1) * CH])
                    xt_sb[hb, ch] = t

            ai = 0
            for _ in range(repeat):
                for gi, grp in enumerate(groups):
                    py = ps_y.tile([P, CH], FP32, tag="py")
                    if ai < 4 and stages == "full":
                        # first use of each ring bank: zero the lines
                        # the matmuls never touch so the full-width DMA
                        # ships only finite values (power-on PSUM may
                        # hold NaN)
                        nc.vector.memset(py[:], 0.0)
                    ai += 1
                    for k, ch in enumerate(grp):
                        for r in range(2):
                            for j in range(4):
                                hb = 4 * r + j
                                nc.tensor.matmul(
                                    py[32 * j:32 * j + gsz, :],
                                    lhsT=wp_sb[:, hb, k, 0:gsz],
                                    rhs=xt_sb[hb, ch][:],
                                    start=(k == 0 and r == 0),
                                    stop=(k == len(grp) - 1 and r == 1),
                                    tile_position=(0, 32 * j),
                                )
                    if stages == "mm":
                        continue
                    ysg = vecs.tile([P, CH], BF16, tag="ysg")
                    # one evacuation per group, alternating DVE/ACT
                    # (Pool/GPSIMD cannot access PSUM), and one
                    # full-width DMA per group on alternating HW DGE
                    # queues; line 32j+k is chunk grp[k]'s partial j
                    if gi % 2 == 0:
                        nc.vector.tensor_copy(ysg[:], py[:])
                    else:
                        nc.scalar.copy(ysg[:], py[:])
                    dq = nc.sync if gi % 2 == 0 else nc.scalar
                    dq.dma_start(y4_d.ap()[gi, :, :], ysg[:])

    nc.compile()
    return nc


def _seg_mask(attn, mlm):
    """(B, C, L) segment mask, exactly as the reference builds it."""
    idx = np.arange(L)
    marker = np.where(mlm > 0, idx[None, :], L)
    starts = np.sort(marker, axis=1)[:, :C]
    end_idx = attn.sum(axis=1)
    bounds = np.concatenate([starts[:, 1:] - 1, (end_idx - 1)[:, None]],
                            axis=1)
    return ((idx[None, None, :] >= starts[:, :, None] + 1)
            & (idx[None, None, :] < bounds[:, :, None]))


def _host_att(X, attn, seg, Wh, bh, q, scale):
    """Per-row seg-pooled attention S_att (B, C).

    Saturated rows (one padded token with probs <= -0.04, i.e. max
    logit >= 40 vs the segment tokens' 0) have S_att < 1e-14 -> 0.
    Rows with no padding have z == 0 everywhere -> uniform att.  Any
    other row gets an exact host softmax.
    """
    S_att = np.zeros((B, C), np.float32)
    for b in range(B):
        pad = np.nonzero(attn[b] == 0)[0]
        if len(pad) == 0:
            S_att[b] = seg[b].sum(axis=1, dtype=np.float64) / L
            continue
        samp = pad[:32]
        pr = np.tanh(X[b, samp] @ Wh.T + bh) @ q * scale
        if pr.min() <= -0.04:
            continue  # saturated: S_att stays 0
        pr = np.tanh(X[b, pad] @ Wh.T + bh) @ q * scale
        z = np.zeros(L)
        z[pad] = -1000.0 * pr
        z -= z.max()
        e = np.exp(z)
        S_att[b] = (seg[b] @ (e / e.sum())).astype(np.float32)
    return S_att


def prep_inputs(inputs):
    """Full inputs -> (per-core in_maps, host epilogue context)."""
    X = np.ascontiguousarray(np.asarray(inputs["input"], dtype=np.float32))
    attn = np.asarray(inputs["attention_mask"])
    mlm = np.asarray(inputs["mlm_mask"])
    Wh = np.asarray(inputs["W_hidden"], dtype=np.float32)
    bh = np.asarray(inputs["b_hidden"], dtype=np.float32)
    q = np.asarray(inputs["query"], dtype=np.float32)[:, 0]
    Wc = np.asarray(inputs["W_cls"], dtype=np.float32)[0]
    bc = float(np.asarray(inputs["b_cls"], dtype=np.float32)[0])

    qvar = np.var(q.astype(np.float64), ddof=1)
    scale = 1.0 / np.sqrt(A * qvar)

    wcT = Wc.reshape(HB, P).T                      # [P, HB]
    wp = np.zeros((P, HB, 8, 8), np.float32)       # shifted variants
    for k in range(8):
        wp[:, :, k, k] = wcT
    wp = wp.reshape(P, HB * 8 * 8).astype(ml_dtypes.bfloat16)

    # --- compact gather of segment tokens (the only ones Sy pools) ---
    seg = _seg_mask(attn, mlm)
    need_y = seg.any(axis=1)
    yb, yt = np.nonzero(need_y)
    t_y = len(yb)
    ycap = CH * max(1, min(MAX_YCH, int(np.ceil(t_y / (N_CORES * CH)))))
    n_ydev = min(t_y, N_CORES * ycap)
    yslots = np.zeros(N_CORES * ycap, np.int64)
    yslots[:n_ydev] = (yb * L + yt)[:n_ydev]

    Xf = X.reshape(B * L, H)
    in_maps = []
    for c in range(N_CORES):
        ycols = yslots[c * ycap:(c + 1) * ycap]
        # xt[hb, p, t] = X[ycols[t], hb*128+p]
        xt_c = np.ascontiguousarray(
            Xf[ycols].T.reshape(HB, P, ycap)).astype(ml_dtypes.bfloat16)
        in_maps.append(dict(xt=xt_c, wp=wp))

    S_att = _host_att(X, attn, seg, Wh, bh, q, scale)
    host_ctx = dict(seg=seg, S_att=S_att, Wc=Wc, bc=bc, X=X, ycap=ycap,
                    n_ydev=n_ydev, yb=yb, yt=yt)
    return in_maps, host_ctx


def epilogue(y, ctx):
    """Host: segment pooling + rank-1 classifier recombination."""
    seg = ctx["seg"].astype(np.float32)
    Sy = np.einsum("bcl,bl->bc", seg, y)
    S_att = ctx["S_att"]
    Wsum = ctx["Wc"].sum(dtype=np.float32)
    return (S_att * Wsum + Sy + ctx["bc"]).astype(np.float32)[:, :, None]


_prog_cache = {}


def kernel(**inputs) -> np.ndarray:
    in_maps, ctx = prep_inputs(inputs)
    key = (ctx["ycap"],)
    if key not in _prog_cache:
        _prog_cache[key] = build_program(ycap=ctx["ycap"])
    nc = _prog_cache[key]
    res = run_bass_kernel_spmd(nc, in_maps, core_ids=list(range(N_CORES)))
    ych = ctx["ycap"] // CH
    percore = []
    for c in range(N_CORES):
        y4g = res.results[c]["y4"]            # [ngrp, 128, CH] bf16
        # line 32j+b of group g = tile-j partial of chunk 4g+b
        g4 = y4g.reshape(y4g.shape[0], 4, 32, CH).astype(np.float32)
        yc = g4[:, :, 0:4, :].sum(axis=1)     # [ngrp, 4, CH] over tiles
        percore.append(yc.reshape(-1, CH)[:ych].reshape(-1))
    yflat = np.concatenate(percore)
    n_ydev, yb, yt = ctx["n_ydev"], ctx["yb"], ctx["yt"]
    y = np.zeros((B, L), np.float32)
    y[yb[:n_ydev], yt[:n_ydev]] = yflat[:n_ydev]
    if n_ydev < len(yb):  # y leftover beyond device capacity, on host
        lb, lt = yb[n_ydev:], yt[n_ydev:]
        y[lb, lt] = (ctx["X"][lb, lt] @ ctx["Wc"]).astype(np.float32)
    return epilogue(y, ctx)
